# revision 3
# baseline (speedup 1.0000x reference)
"""Trainium2 Bass kernel for nn_CandidateFinder (retrieval_knn), v2.

Algorithm (per batch b): pack each row's 8 sign bits into a code in [0,256).
For query i the output row is the first min(m,64) key indices j with
k_code[j]==q_code[i], ascending, left-padded with -1 to 64.

Device mapping: 8 cores = 4 batches x 2 code-groups. Core (b, cg) handles all
4096 keys and all 4096 queries of batch b, but only codes [cg*128, cg*128+128)
(code c lives on partition c-cg*128):
 - keys are fed in DESCENDING j order; per-chunk forward inclusive running
   counts (tensor_tensor_scan from 0, plus per-partition chunk offsets) give
   each key its descending rank r within its code bucket.
 - local_scatter places value (j+1) at slot r-1 of its code's table row,
   one call per 1024-key chunk into its own table (slots are globally unique
   across chunks), merged by adds; the GPSIMD scatters pipeline behind the
   DVE scans.
 - table rows then hold [j_max+1, j_2nd+1, ..., 0 pads]; reading slots 63..0
   gives [0 pads ..., ascending j+1], split into hi=(v>>6) and lo=(v&63) bf16
   planes so a one-hot query-code matmul (PE) gathers rows exactly in bf16.
 - out_i32[i, u] = 64*hi + lo = (j+1) or 0. Host combines the two code-group
   cores with out = (a + b) - 1: exactly one core matches each query's code,
   the other contributes 0, and pads become -1. All host work (key reversal,
   final add) is data-independent.

Assumes per-(batch, code) bucket size <= 128 (larger buckets would overflow
the scatter scratch); the reference harness data has max bucket 29 and the
test asserts <= 64.
"""

import os
import sys

for _p in ("/opt/trn_rl_repo", "/root/.axon_site/_ro/trn_rl_repo"):
    if os.path.isdir(_p) and _p not in sys.path:
        sys.path.insert(0, _p)

import numpy as np

from concourse import bacc, bass, mybir, tile
from concourse import bass_utils

F32 = mybir.dt.float32
BF16 = mybir.dt.bfloat16
I32 = mybir.dt.int32
I16 = mybir.dt.int16
ALU = mybir.AluOpType

B, L, D, KMAX = 4, 4096, 8, 64
NCH = 4                      # scan/scatter chunks
CH = L // NCH                # 1024 keys per chunk
QB = L // 128                # 32 query blocks of 128


def build_nc():
    nc = bacc.Bacc("TRN2", target_bir_lowering=False)

    keys = nc.dram_tensor("keys", [L, D], F32, kind="ExternalInput")
    queries = nc.dram_tensor("queries", [L, D], F32, kind="ExternalInput")
    pw = nc.dram_tensor("pw", [128, 32 * 8], F32, kind="ExternalInput")
    cpart = nc.dram_tensor("cpart", [128, 1], F32, kind="ExternalInput")
    vals_h = nc.dram_tensor("vals_h", [128, L], I16, kind="ExternalInput")
    qhbm = nc.dram_tensor("qhbm", [L], BF16, kind="Internal")
    out = nc.dram_tensor("out", [L, KMAX], I32, kind="ExternalOutput")

    with tile.TileContext(nc) as tc:
        with (
            tc.tile_pool(name="sb", bufs=1) as sb,
            tc.tile_pool(name="ps", bufs=1, space="PSUM") as ps,
        ):
            # ---- inputs (kfeat first: it unblocks all key-side compute; the
            # 1 MB vals const is not needed until the first scatter) ----
            kfeat = sb.tile([128, 256], F32, tag="kfeat")
            nc.sync.dma_start(
                kfeat[:], keys.ap().rearrange("(p t) d -> p (t d)", p=128))
            pwt = sb.tile([128, 256], F32, tag="pwt")
            nc.sync.dma_start(pwt[:], pw.ap())
            cp = sb.tile([128, 1], F32, tag="cp")
            nc.sync.dma_start(cp[:], cpart.ap())
            qfeat = sb.tile([128, 256], F32, tag="qfeat")
            nc.sync.dma_start(
                qfeat[:], queries.ap().rearrange("(p t) d -> p (t d)", p=128))

            qbp = sb.tile([128, 256], BF16, tag="qbp")
            nc.vector.scalar_tensor_tensor(
                qbp[:], qfeat[:], 0.0, pwt[:], ALU.is_gt, ALU.mult)
            qcode = sb.tile([128, 32], BF16, tag="qcode")
            with nc.allow_low_precision(reason="codes <= 255, exact in bf16"):
                nc.vector.tensor_reduce(
                    qcode[:], qbp[:].rearrange("p (a b) -> p a b", b=8),
                    axis=mybir.AxisListType.X, op=ALU.add)

            nc.sync.dma_start(
                bass.AP(qhbm, 0, [[32, 128], [1, 32]]), qcode[:])
            qcodeB = sb.tile([128, L], BF16, tag="qcodeB")
            nc.sync.dma_start(qcodeB[:], bass.AP(qhbm, 0, [[0, 128], [1, L]]))

            kbp = sb.tile([128, 256], BF16, tag="kbp")
            nc.vector.scalar_tensor_tensor(
                kbp[:], kfeat[:], 0.0, pwt[:], ALU.is_gt, ALU.mult)
            kcode = sb.tile([128, 32], BF16, tag="kcode")
            with nc.allow_low_precision(reason="codes <= 255, exact in bf16"):
                nc.vector.tensor_reduce(
                    kcode[:], kbp[:].rearrange("p (a b) -> p a b", b=8),
                    axis=mybir.AxisListType.X, op=ALU.add)

            # broadcast key codes to all partitions on-chip: flatten to one
            # partition (SBUF->SBUF DMA, Act queue), then a K=1 ones-matmul
            # streams the row into PSUM across all 128 partitions -- much
            # lower latency than an HBM write + broadcast-read round trip.
            krow = sb.tile([1, L], BF16, tag="krow")
            nc.scalar.dma_start(krow[:], kcode[:])
            ones1 = sb.tile([1, 128], BF16, tag="ones1")
            nc.vector.memset(ones1[:], 1.0)
            psum = ps.tile([128, L], F32, tag="psum")
            for m in range(L // 512):
                nc.tensor.matmul(
                    psum[:, m * 512:(m + 1) * 512],
                    ones1[:], krow[:, m * 512:(m + 1) * 512],
                    start=True, stop=True)
            # DVE reads PSUM at ~3x the cost of SBUF: stage the broadcast
            # through SBUF on the otherwise-idle Act engine, per chunk
            kcodeB = sb.tile([128, L], BF16, tag="kcodeB")
            # 1 MB of scatter values, first needed ~15us in: keep it behind
            # the latency-critical DMAs so it never blocks them
            vals = sb.tile([128, L], I16, tag="vals")
            nc.scalar.dma_start(vals[:], vals_h.ap())

            # ---- chunked: match -> rank scan -> slot idx -> local scatter ----
            # slot = global descending rank - 1; unmatched keys get idx -1 so
            # the Q7 scatter predicates them off (writing them all to a trash
            # slot serializes its vector scatter on address conflicts).
            # Ranks above 128 would overflow the scatter scratch; harness
            # buckets are <= 29 (test asserts <= 64).
            k1t = sb.tile([128, L], BF16, tag="k1t")
            cnt = sb.tile([128, L], BF16, tag="cnt")
            s1 = sb.tile([128, L], BF16, tag="s1")
            t0 = sb.tile([128, L], BF16, tag="t0")
            idx = sb.tile([128, L], I16, tag="idx")
            offs = sb.tile([128, NCH], F32, tag="offs")
            nc.vector.memset(offs[:], 0)
            NELEM = 128
            tbls = [sb.tile([128, NELEM], I16, name=f"tblc{k}", tag=f"tblc{k}")
                    for k in range(NCH)]
            for k in range(NCH):
                sl = slice(k * CH, (k + 1) * CH)
                end = slice((k + 1) * CH - 1, (k + 1) * CH)
                if k == 0:
                    nc.vector.tensor_scalar(
                        k1t[:, sl], psum[:, sl], cp[:], None, ALU.is_equal)
                else:
                    nc.scalar.copy(kcodeB[:, sl], psum[:, sl])
                    nc.vector.tensor_scalar(
                        k1t[:, sl], kcodeB[:, sl], cp[:], None, ALU.is_equal)
                nc.vector.tensor_tensor_scan(
                    cnt[:, sl], k1t[:, sl], k1t[:, sl], 0.0,
                    ALU.add, ALU.bypass)
                if k + 1 < NCH:
                    # off_{k+1} = off_k + chunk_total_k (per-partition)
                    nc.vector.scalar_tensor_tensor(
                        offs[:, k + 1:k + 2], cnt[:, end], 0.0,
                        offs[:, k:k + 1], ALU.add, ALU.add)
                nc.vector.tensor_scalar(
                    s1[:, sl], cnt[:, sl], offs[:, k:k + 1], None, ALU.add)
                nc.vector.tensor_tensor(
                    t0[:, sl], s1[:, sl], k1t[:, sl], ALU.mult)
                nc.vector.tensor_scalar(
                    idx[:, sl], t0[:, sl], -1.0, None, ALU.add)
                nc.gpsimd.local_scatter(
                    out_ap=tbls[k][:], data_ap=vals[:, sl], idxs_ap=idx[:, sl],
                    channels=128, num_elems=NELEM, num_idxs=CH)

            # ---- query one-hot (DVE fills this in while GPSIMD scatters) ----
            q1t = sb.tile([128, L], BF16, tag="q1t")
            for k in range(NCH):
                sl = slice(k * CH, (k + 1) * CH)
                nc.vector.tensor_scalar(
                    q1t[:, sl], qcodeB[:, sl], cp[:], None, ALU.is_equal)

            # ---- merge chunk tables (slots 0..63 are all we read later) ----
            MW = KMAX
            m0 = sb.tile([128, MW], I16, tag="m0")
            m1 = sb.tile([128, MW], I16, tag="m1")
            tbl = sb.tile([128, MW], I16, tag="tbl")
            nc.vector.tensor_tensor(
                m0[:], tbls[0][:, 0:MW], tbls[1][:, 0:MW], ALU.add)
            nc.vector.tensor_tensor(
                m1[:], tbls[2][:, 0:MW], tbls[3][:, 0:MW], ALU.add)
            nc.vector.tensor_tensor(tbl[:], m0[:], m1[:], ALU.add)

            # ---- hi/lo bf16 planes, reversed so col u = slot 64-u ----
            # (shift/and only codegen on 32-bit: widen the reversed table)
            _pp = list(tbl[:].ap[0])
            trev = bass.AP(tbl.tensor, KMAX - 1, [_pp, [-1, KMAX]])
            t32 = sb.tile([128, KMAX], I32, tag="t32")
            nc.scalar.copy(t32[:], trev)
            hi32 = sb.tile([128, KMAX], I32, tag="hi32")
            nc.vector.tensor_scalar(hi32[:], t32[:], 6, None, ALU.arith_shift_right)
            lo32 = sb.tile([128, KMAX], I32, tag="lo32")
            nc.vector.tensor_scalar(lo32[:], t32[:], 63, None, ALU.bitwise_and)
            tbl2 = sb.tile([128, 2 * KMAX], BF16, tag="tbl2")
            nc.scalar.copy(tbl2[:, 0:KMAX], hi32[:])
            nc.scalar.copy(tbl2[:, KMAX:2 * KMAX], lo32[:])

            # ---- one-hot gather matmul (reuses the broadcast PSUM tile) ----
            for m in range(QB):
                nc.tensor.matmul(
                    psum[:, m * 128:(m + 1) * 128],
                    q1t[:, m * 128:(m + 1) * 128],
                    tbl2[:],
                    start=True, stop=True)

            # ---- combine 64*hi+lo and store, split in halves for overlap ----
            # (stt can't read two PSUM operands: stage lo through SBUF on Act)
            o32 = sb.tile([128, QB * KMAX], I32, tag="o32")
            loS = sb.tile([128, QB * KMAX], BF16, tag="loS")
            p3 = psum[:].rearrange("p (m u) -> p m u", u=128)
            o3 = o32[:].rearrange("p (m u) -> p m u", u=KMAX)
            l3 = loS[:].rearrange("p (m u) -> p m u", u=KMAX)
            # HBM row r = p*QB + m (contiguous per partition: 128 descriptors
            # per store instead of 4096); the host maps row r back to query
            # i = (r % QB) * 128 + r // QB.
            H = QB // 4
            for h in range(4):
                ms = slice(h * H, (h + 1) * H)
                nc.scalar.copy(l3[:, ms], p3[:, ms, KMAX:2 * KMAX])
                nc.vector.scalar_tensor_tensor(
                    o3[:, ms], p3[:, ms, 0:KMAX], 64.0,
                    l3[:, ms], ALU.mult, ALU.add)
                nc.sync.dma_start(
                    bass.AP(out, h * H * KMAX,
                            [[QB * KMAX, 128], [KMAX, H], [1, KMAX]]),
                    o3[:, ms])
    return nc


_NC_CACHE = None


def _get_nc():
    global _NC_CACHE
    if _NC_CACHE is None:
        nc = build_nc()
        nc.compile()
        _NC_CACHE = nc
    return _NC_CACHE


def _consts():
    pwv = np.tile((2.0 ** np.arange(8, dtype=np.float32))[None, :], (128, 32))
    vals = np.tile((L - np.arange(L, dtype=np.int16))[None, :], (128, 1))
    cparts = [
        (cg * 128 + np.arange(128, dtype=np.float32)).reshape(128, 1)
        for cg in range(2)
    ]
    return pwv, vals, cparts


def _make_in_maps(query_up, key_up):
    pwv, vals, cparts = _consts()
    in_maps = []
    for core in range(8):
        b, cg = core // 2, core % 2
        in_maps.append({
            "keys": np.ascontiguousarray(key_up[b, ::-1]),
            "queries": np.ascontiguousarray(query_up[b]),
            "pw": pwv,
            "cpart": cparts[cg],
            "vals_h": vals,
        })
    return in_maps


def kernel(query_up, key_up, head_idx=None, **_ignored):
    query_up = np.asarray(query_up, dtype=np.float32)
    key_up = np.asarray(key_up, dtype=np.float32)
    nc = _get_nc()
    in_maps = _make_in_maps(query_up, key_up)
    res = bass_utils.run_bass_kernel_spmd(nc, in_maps, core_ids=list(range(8)))
    r = np.arange(L)
    iofr = (r % QB) * 128 + r // QB
    out = np.empty((B, L, KMAX), dtype=np.int64)
    for b in range(B):
        a = res.results[2 * b]["out"].astype(np.int64)
        c = res.results[2 * b + 1]["out"]
        out[b, iofr] = a + c - 1
    return out


def run_profiled(query_up, key_up, head_idx=None, **_ignored):
    query_up = np.asarray(query_up, dtype=np.float32)
    key_up = np.asarray(key_up, dtype=np.float32)
    nc = _get_nc()
    in_maps = _make_in_maps(query_up, key_up)
    return bass_utils.run_bass_kernel_spmd(
        nc, in_maps, core_ids=list(range(8)), trace=True)
